# revision 3
# baseline (speedup 1.0000x reference)
"""KNN anomaly-score kernel for Trainium2 (8 NeuronCores, Bass/Tile).

Problem: features [B=1024, D=768], memory_bank [N=50000, D=768], k=9.
anomaly_score[b] = mean of the k smallest Euclidean distances from
features[b] to the memory bank rows.

Strategy (per the sharding hint): shard memory-bank rows across the 8
cores.  Each core computes its [B, N/8] block of v = -d^2/2 =
f.m - |m|^2/2 - |f|^2/2 on the TensorEngine: the GEMM runs in fp8-e4m3
with the DoubleRow perf mode (two K=128 subtiles reduced per
instruction -> 2x bf16 throughput), while the norm terms are folded in
exactly via a K=4 bf16 augmented matmul whose constants are split hi/lo
across two bf16 rows (compensated summation), accumulated in fp32 PSUM.
The four augment matmuls of a chunk pair run concurrently in disjoint
32-row PE groups (tile_position), so they cost ~one matmul slot.

Selection: for each 1024-column block the DVE MAX8 instruction extracts
the block's top-8 v values (one pass, no match_replace).  The device
returns all block candidates [B, 8*nblocks]; the host gathers the 8
cores' candidates and reduces to the global top-k.  A true top-k member
can be missing only if >=8 elements of its block rank above it, which
forces >=8 of the observed top-k to come from that single block - the
host detects exactly that condition and recomputes the affected rows
with numpy, so gross selection failures are corrected for any k.
"""

import functools
import sys

sys.path.insert(0, "/opt/trn_rl_repo")

import numpy as np

P = 128
NCORES = 8
PAD_VAL = -1.0e30  # v-value of padding columns (never selected)


def _ceil_to(x, m):
    return (x + m - 1) // m * m


@functools.lru_cache(maxsize=4)
def _build(B, D, NPAD):
    """Build (and finalize) the SPMD Bass module for one core's shard."""
    from contextlib import ExitStack

    import concourse.tile as tile
    from concourse import bacc, mybir

    f32 = mybir.dt.float32
    bf16 = mybir.dt.bfloat16
    fp8 = mybir.dt.float8e4
    DR = mybir.MatmulPerfMode.DoubleRow

    KT = D // P
    MT = B // P
    assert D % P == 0 and B % P == 0 and NPAD >= 1024
    assert KT % 2 == 0, "DoubleRow needs an even number of K tiles"
    KTP = KT // 2
    # process blocks of 1024 columns (one 2-bank PSUM tile), ragged tail
    chunks = []
    c0 = 0
    while c0 < NPAD:
        w = min(1024, NPAD - c0)
        rem = NPAD - c0 - w
        if 0 < rem < 8:
            w -= 8 - rem  # keep the next (last) chunk MAX8-legal (>=8)
        chunks.append((c0, w))
        c0 += w
    NCH = len(chunks)
    CW = 8 * NCH  # candidates per row per core

    nc = bacc.Bacc(
        "TRN2", target_bir_lowering=False, debug=False, num_devices=NCORES
    )

    f_t = nc.declare_dram_parameter("f_t", [D, B], fp8, isOutput=False)
    aug_l = nc.declare_dram_parameter("aug_l", [4, B], bf16, isOutput=False)
    b_t = nc.declare_dram_parameter("b_t", [D, NPAD], fp8, isOutput=False)
    aug_r = nc.declare_dram_parameter("aug_r", [4, NPAD], bf16, isOutput=False)
    out = nc.declare_dram_parameter("cand", [B, CW], f32, isOutput=True)

    with tile.TileContext(nc) as tc, ExitStack() as ctx:
        cpool = ctx.enter_context(tc.tile_pool(name="const", bufs=1))
        bpool = ctx.enter_context(tc.tile_pool(name="bank", bufs=6))
        ppool = ctx.enter_context(tc.tile_pool(name="psum", bufs=4, space="PSUM"))
        upool = ctx.enter_context(tc.tile_pool(name="u", bufs=6))

        # group K tiles in pairs: DoubleRow consumes [K, 2, .] slices
        b_t_view = b_t.rearrange("(kp two p) n -> p kp two n", p=P, two=2)
        f_t_view = f_t.rearrange("(kp two p) b -> p kp two b", p=P, two=2)

        # PE warm-up during the initial DMA wait: garbage matmuls on a
        # zeroed tile get the HAM clock-gate to 2.4GHz before real work
        warm = cpool.tile([P, 512], bf16, tag="warm")
        nc.vector.memset(warm[:], 0.0)
        wpsum = ppool.tile([P, 1024], f32, tag="pt")  # borrow a pt slot
        for _ in range(6):
            nc.tensor.matmul(
                wpsum[:, :512], lhsT=warm[:, :P], rhs=warm[:], start=True, stop=True
            )

        # per-ktp tiles + interleaved DMAs so the first matmuls can start
        # as soon as the ktp=0 slices land (instead of after one huge DMA)
        ftiles = [
            cpool.tile([P, 2, B], fp8, tag=f"ft{kp}", name=f"ft{kp}")
            for kp in range(KTP)
        ]
        bt0 = [
            bpool.tile([P, 2, 1024], fp8, tag=f"bt0_{kp}", name=f"bt0_{kp}")
            for kp in range(KTP)
        ]
        augl_t = cpool.tile([P, B], bf16, tag="augl")
        augr_t = cpool.tile([P, NPAD], bf16, tag="augr")
        W0 = chunks[0][1]
        nc.sync.dma_start(bt0[0][:, :, :W0], b_t_view[:, 0, :, :W0])
        nc.sync.dma_start(ftiles[0][:], f_t_view[:, 0, :, :])
        # augment rows replicated at partition bases {0,32,64,96} so four
        # K=4 augment matmuls can run concurrently in disjoint PE row groups.
        # These are tiny; issue them on the fast 16-engine sync queue right
        # after the first k-pair slices (the scalar queue delivered them
        # ~15us late, stalling the whole psum pipeline behind the augments).
        for j in range(4):
            nc.sync.dma_start(augl_t[32 * j : 32 * j + 4, :], aug_l[:])
            nc.sync.dma_start(augr_t[32 * j : 32 * j + 4, :], aug_r[:])
        for kp in range(1, KTP):
            nc.sync.dma_start(bt0[kp][:, :, :W0], b_t_view[:, kp, :, :W0])
            nc.sync.dma_start(ftiles[kp][:], f_t_view[:, kp, :, :])

        # enqueue every later bank chunk's load up front; the pool's slots
        # gate the actual transfers
        btiles = {}
        for ci, (c0, W) in enumerate(chunks):
            if ci == 0:
                continue
            btile = bpool.tile([P, KTP, 2, 1024], fp8, tag="bt", name=f"bt{ci}")
            nc.sync.dma_start(btile[:, :, :, :W], b_t_view[:, :, :, c0 : c0 + W])
            btiles[ci] = btile

        cand_tiles = [
            cpool.tile([P, CW], f32, tag=f"cand{m}", name=f"cand{m}")
            for m in range(MT)
        ]

        def bslice(ci2, kp, lo, w):
            if ci2 == 0:
                return bt0[kp][:, :, lo : lo + w]
            return btiles[ci2][:, kp, :, lo : lo + w]

        def chalves(W):
            out_, lo = [], 0
            while lo < W:
                out_.append((lo, min(512, W - lo)))
                lo += 512
            return out_

        # process full chunks in pairs: each lhsT load feeds 4 consecutive
        # matmuls, and the pair's 4 K=4 augment matmuls pack into ONE
        # concurrent 4-way row-group slot
        pairs = []
        ci = 0
        while ci < NCH:
            if (
                ci > 0  # chunk 0 alone: its DMA gates kernel start
                and ci + 1 < NCH
                and len(chalves(chunks[ci][1]))
                + len(chalves(chunks[ci + 1][1]))
                <= 4  # augments must fit the 4 PE row groups
            ):
                pairs.append((ci, ci + 1))
                ci += 2
            else:
                pairs.append((ci,))
                ci += 1

        for pair in pairs:
            for m in range(MT):
                pts = {}
                for ci2 in pair:
                    pts[ci2] = ppool.tile([P, 1024], f32, tag="pt", name=f"pt{ci2}_{m}")
                for kp in range(KTP):
                    for ci2 in pair:
                        c0, W = chunks[ci2]
                        for hlo, hw in chalves(W):
                            nc.tensor.matmul(
                                pts[ci2][:, hlo : hlo + hw],
                                lhsT=ftiles[kp][:, :, m * P : (m + 1) * P],
                                rhs=bslice(ci2, kp, hlo, hw),
                                start=(kp == 0),
                                stop=False,
                                perf_mode=DR,
                            )
                j = 0
                for ci2 in pair:
                    c0, W = chunks[ci2]
                    for hlo, hw in chalves(W):
                        nc.tensor.matmul(
                            pts[ci2][:, hlo : hlo + hw],
                            lhsT=augl_t[
                                32 * j : 32 * j + 4, m * P : (m + 1) * P
                            ],
                            rhs=augr_t[
                                32 * j : 32 * j + 4, c0 + hlo : c0 + hlo + hw
                            ],
                            start=False,
                            stop=True,
                            tile_position=(32 * j, 0),
                        )
                        j += 1
                for ci2 in pair:
                    c0, W = chunks[ci2]
                    u = upool.tile([P, 1024], f32, tag="u")
                    nc.scalar.copy(u[:, :W], pts[ci2][:, :W])
                    nc.vector.max(
                        cand_tiles[m][:, ci2 * 8 : ci2 * 8 + 8], u[:, :W]
                    )

        for m in range(MT):
            nc.sync.dma_start(out[m * P : (m + 1) * P, :], cand_tiles[m][:])

    nc.finalize()
    return nc


def _split_bf16(x):
    """hi/lo bf16 split of a float32 vector: hi + lo ~= x to ~2^-17."""
    import ml_dtypes

    bf = ml_dtypes.bfloat16
    hi = x.astype(bf)
    lo = (x - hi.astype(np.float32)).astype(bf)
    return hi, lo


def _host_prep(features, memory_bank):
    """Shard + lay out inputs for the 8 cores."""
    import ml_dtypes

    bf = ml_dtypes.bfloat16
    f8 = ml_dtypes.float8_e4m3
    B, D = features.shape
    N = memory_bank.shape[0]
    NSH = -(-N // NCORES)
    NPAD = max(NSH, 1024)
    if NPAD % 1024 and NPAD % 1024 < 8:
        NPAD = _ceil_to(NPAD, 1024)  # keep the ragged tail MAX8-legal (>=8)

    fT = np.ascontiguousarray(features.T).astype(f8)
    x_sq = np.einsum("bd,bd->b", features, features, dtype=np.float32)
    xh, xl = _split_bf16(-0.5 * x_sq)
    augL = np.empty((4, B), bf)
    augL[0] = 1.0
    augL[1] = 1.0
    augL[2] = xh
    augL[3] = xl

    msq = np.einsum("nd,nd->n", memory_bank, memory_bank, dtype=np.float32)

    in_maps = []
    for i in range(NCORES):
        lo = i * NSH
        hi = min(lo + NSH, N)
        n_i = hi - lo
        if n_i == NPAD:
            bT = np.ascontiguousarray(memory_bank[lo:hi].T).astype(f8)
        else:
            bT = np.zeros((D, NPAD), f8)
            bT[:, :n_i] = memory_bank[lo:hi].T.astype(f8)
        mh, ml = _split_bf16(-0.5 * msq[lo:hi])
        augR = np.zeros((4, NPAD), bf)
        augR[0] = PAD_VAL
        augR[0, :n_i] = mh
        augR[1, :n_i] = ml
        augR[2] = 1.0
        augR[3] = 1.0
        in_maps.append({"f_t": fT, "aug_l": augL, "b_t": bT, "aug_r": augR})
    return in_maps, NPAD, x_sq, msq


# test.py can flip these to get a profiled run
TRACE = False
LAST_RESULT = None
N_RECOMPUTED = 0


def _install_ntff_hook():
    """This container's `antenv` lacks `axon_hooks`; synthesize it so
    run_bass_kernel_spmd(trace=True) can profile via the axon .so."""
    import sys as _sys

    if "antenv.axon_hooks" in _sys.modules:
        return
    import contextlib, ctypes, types

    mod = types.ModuleType("antenv.axon_hooks")
    mod._hook = None
    mod.set_axon_ntff_profile_hook = lambda h: setattr(mod, "_hook", h)
    mod.get_axon_ntff_profile_hook = lambda: mod._hook

    so_path = "/opt/axon/libaxon_pjrt.so"
    try:
        lib = ctypes.CDLL(so_path)
        lib.axon_start_nrt_profile.argtypes = [
            ctypes.POINTER(ctypes.c_int64),
            ctypes.c_size_t,
        ]
        lib.axon_start_nrt_profile.restype = ctypes.c_int64
        lib.axon_stop_nrt_profile.argtypes = [ctypes.c_char_p]
        lib.axon_stop_nrt_profile.restype = ctypes.c_int64

        @contextlib.contextmanager
        def _hook(output_dir, device_ids):
            import jax

            jax.devices()
            if device_ids:
                ids = (ctypes.c_int64 * len(device_ids))(*device_ids)
                rc = lib.axon_start_nrt_profile(ids, len(device_ids))
            else:
                rc = lib.axon_start_nrt_profile(None, 0)
            if rc != 0:
                raise RuntimeError(f"axon_start_nrt_profile rc={rc}")
            try:
                yield
            finally:
                n = lib.axon_stop_nrt_profile(str(output_dir).encode())
                print(f"profile: {n} file(s) written to {output_dir}")

        mod._hook = _hook
    except (OSError, AttributeError):
        pass

    import antenv

    _sys.modules["antenv.axon_hooks"] = mod
    antenv.axon_hooks = mod


def _exact_row_scores(features, memory_bank, rows, kk):
    """Exact numpy top-k mean distance for a few suspect rows."""
    f = features[rows]  # [R, D]
    d2 = (
        np.einsum("rd,rd->r", f, f)[:, None]
        + np.einsum("nd,nd->n", memory_bank, memory_bank)[None, :]
        - 2.0 * (f @ memory_bank.T)
    )
    d2k = np.sort(d2, axis=1)[:, :kk]
    return np.sqrt(np.maximum(d2k, 0.0)).mean(axis=1)


def kernel(features, memory_bank, k):
    global LAST_RESULT, N_RECOMPUTED
    from concourse.bass_utils import run_bass_kernel_spmd

    features = np.asarray(features, dtype=np.float32)
    memory_bank = np.asarray(memory_bank, dtype=np.float32)
    B, D = features.shape
    N = memory_bank.shape[0]
    kk = min(int(k), N)
    if kk <= 0:
        # mean over an empty candidate set (matches jnp.mean of empty)
        return np.full(B, np.nan, np.float32)

    in_maps, NPAD, x_sq, msq = _host_prep(features, memory_bank)
    nc = _build(B, D, NPAD)

    if TRACE:
        _install_ntff_hook()
    res = run_bass_kernel_spmd(nc, in_maps, list(range(NCORES)), trace=TRACE)
    LAST_RESULT = res

    # gather per-(core, block) top-8 candidates; v = -d^2/2, larger = closer
    v = np.concatenate(
        [res.results[i]["cand"] for i in range(NCORES)], axis=1
    )  # [B, NCORES * 8 * nblocks]
    return _finalize(v, features, memory_bank, kk)


def _finalize(v, features, memory_bank, kk):
    """Reduce the per-(core, block) top-8 candidates to the final scores."""
    global N_RECOMPUTED
    kk_c = min(kk, v.shape[1])
    order = np.argsort(-v, axis=1)[:, :kk_c]  # observed top-k candidates
    vk = np.take_along_axis(v, order, axis=1)
    d = np.sqrt(np.maximum(-2.0 * vk, 0.0))
    scores = d.mean(axis=1).astype(np.float32)

    # A true top-k member can only be missing if >=8 elements of its
    # 1024-column block outrank it; then >=8 of the observed top-k come
    # from that block (index group of 8).  Recompute such rows exactly.
    N_RECOMPUTED = 0
    if kk >= 9:
        if kk > v.shape[1]:  # more than the candidate pool: all rows exact
            suspects = np.arange(v.shape[0])
        else:
            grp = np.sort(order // 8, axis=1)
            same8 = (grp[:, 7:] == grp[:, : grp.shape[1] - 7]).any(axis=1)
            suspects = np.nonzero(same8)[0]
        if suspects.size:
            N_RECOMPUTED = suspects.size
            scores[suspects] = _exact_row_scores(
                features, memory_bank, suspects, kk
            ).astype(np.float32)

    return scores


# revision 9
# speedup vs baseline: 1.2485x; 1.2485x over previous
"""KNN anomaly-score kernel for Trainium2 (8 NeuronCores, Bass/Tile).

Problem: features [B=1024, D=768], memory_bank [N=50000, D=768], k=9.
anomaly_score[b] = mean of the k smallest Euclidean distances from
features[b] to the memory bank rows.

Strategy (per the sharding hint): shard memory-bank rows across the 8
cores.  Each core computes its [B, N/8] block of v = f.m - |m|^2/2 on
the TensorEngine: the GEMM runs in fp8-e4m3 with the DoubleRow perf
mode (two K=128 subtiles reduced per instruction -> 2x bf16
throughput), while the bank-norm term is folded in exactly via a K=2
bf16 augmented matmul whose constants are split hi/lo across two bf16
rows (compensated summation), accumulated in fp32 PSUM.  The |f|^2 term
is a per-row constant - it cannot change the per-row selection - so the
host folds it in exactly afterwards (d^2 = x_sq - 2v).  The augment
matmuls of a chunk pair run concurrently in disjoint 32-row PE groups
(tile_position), so they cost ~one matmul slot; their lhs is an on-chip
memset of ones and their rhs replicas ride the idle GPSIMD DMA queue,
keeping the critical sync-queue DMAs (features + bank) uninterrupted.

Selection: for each 1024-column block the DVE MAX8 instruction extracts
the block's top-8 v values (one pass, no match_replace).  The device
returns all block candidates [B, 8*nblocks]; the host gathers the 8
cores' candidates and reduces to the global top-k.  A true top-k member
can be missing only if >=8 elements of its block rank above it, which
forces >=8 of the observed top-k to come from that single block - the
host detects exactly that condition and recomputes the affected rows
with numpy, so gross selection failures are corrected for any k.
"""

import functools
import sys

sys.path.insert(0, "/opt/trn_rl_repo")

import numpy as np

P = 128
NCORES = 8
PAD_VAL = -1.0e30  # v-value of padding columns (never selected)


def _ceil_to(x, m):
    return (x + m - 1) // m * m


@functools.lru_cache(maxsize=4)
def _build(B, D, NPAD):
    """Build (and finalize) the SPMD Bass module for one core's shard."""
    from contextlib import ExitStack

    import concourse.tile as tile
    from concourse import bacc, mybir

    f32 = mybir.dt.float32
    bf16 = mybir.dt.bfloat16
    fp8 = mybir.dt.float8e4
    DR = mybir.MatmulPerfMode.DoubleRow

    KT = D // P
    MT = B // P
    assert D % P == 0 and B % P == 0 and NPAD >= 1024
    assert KT % 2 == 0, "DoubleRow needs an even number of K tiles"
    KTP = KT // 2
    # process blocks of 1024 columns (one 2-bank PSUM tile), ragged tail
    chunks = []
    c0 = 0
    while c0 < NPAD:
        w = min(1024, NPAD - c0)
        rem = NPAD - c0 - w
        if 0 < rem < 8:
            w -= 8 - rem  # keep the next (last) chunk MAX8-legal (>=8)
        chunks.append((c0, w))
        c0 += w
    NCH = len(chunks)
    CW = 8 * NCH  # candidates per row per core

    nc = bacc.Bacc(
        "TRN2", target_bir_lowering=False, debug=False, num_devices=NCORES
    )

    f_t = nc.declare_dram_parameter("f_t", [D, B], fp8, isOutput=False)
    b_t = nc.declare_dram_parameter("b_t", [D, NPAD], fp8, isOutput=False)
    aug_r = nc.declare_dram_parameter("aug_r", [2, NPAD], bf16, isOutput=False)
    out = nc.declare_dram_parameter("cand", [B, CW], f32, isOutput=True)

    with tile.TileContext(nc) as tc, ExitStack() as ctx:
        cpool = ctx.enter_context(tc.tile_pool(name="const", bufs=1))
        bpool = ctx.enter_context(tc.tile_pool(name="bank", bufs=6))
        ppool = ctx.enter_context(tc.tile_pool(name="psum", bufs=4, space="PSUM"))
        upool = ctx.enter_context(tc.tile_pool(name="u", bufs=6))

        # group K tiles in pairs: DoubleRow consumes [K, 2, .] slices
        b_t_view = b_t.rearrange("(kp two p) n -> p kp two n", p=P, two=2)
        f_t_view = f_t.rearrange("(kp two p) b -> p kp two b", p=P, two=2)

        # PE warm-up during the initial DMA wait: garbage matmuls on a
        # zeroed tile get the HAM clock-gate to 2.4GHz before real work
        warm = cpool.tile([P, 512], bf16, tag="warm")
        nc.vector.memset(warm[:], 0.0)
        wpsum = ppool.tile([P, 1024], f32, tag="pt")  # borrow a pt slot
        for _ in range(6):
            nc.tensor.matmul(
                wpsum[:, :512], lhsT=warm[:, :P], rhs=warm[:], start=True, stop=True
            )

        # per-ktp tiles + interleaved DMAs so the first matmuls can start
        # as soon as the ktp=0 slices land (instead of after one huge DMA)
        ftiles = [
            cpool.tile([P, 2, B], fp8, tag=f"ft{kp}", name=f"ft{kp}")
            for kp in range(KTP)
        ]
        bt0 = [
            bpool.tile([P, 2, 1024], fp8, tag=f"bt0_{kp}", name=f"bt0_{kp}")
            for kp in range(KTP)
        ]
        augl_t = cpool.tile([P, B], bf16, tag="augl")
        augr_t = cpool.tile([P, NPAD], bf16, tag="augr")
        W0 = chunks[0][1]
        for kp in range(KTP):
            nc.sync.dma_start(bt0[kp][:, :, :W0], b_t_view[:, kp, :, :W0])
            nc.sync.dma_start(ftiles[kp][:], f_t_view[:, kp, :, :])
        # augment rows replicated at partition bases {0,32,64,96} so four
        # K=2 augment matmuls can run concurrently in disjoint PE row groups.
        # The ones-vector lhs is memset on-chip (no DMA), and the rhs
        # replicas go on the otherwise-idle GPSIMD DMA queue so the critical
        # sync-queue stream (features + bank chunks) is never interrupted.
        for j in range(4):
            nc.gpsimd.memset(augl_t[32 * j : 32 * j + 2, :], 1.0)
            nc.gpsimd.dma_start(augr_t[32 * j : 32 * j + 2, :], aug_r[:])

        # enqueue every later bank chunk's load up front; the pool's slots
        # gate the actual transfers
        btiles = {}
        for ci, (c0, W) in enumerate(chunks):
            if ci == 0:
                continue
            btile = bpool.tile([P, KTP, 2, 1024], fp8, tag="bt", name=f"bt{ci}")
            nc.sync.dma_start(btile[:, :, :, :W], b_t_view[:, :, :, c0 : c0 + W])
            btiles[ci] = btile

        cand_tiles = [
            cpool.tile([P, CW], f32, tag=f"cand{m}", name=f"cand{m}")
            for m in range(MT)
        ]

        def bslice(ci2, kp, lo, w):
            if ci2 == 0:
                return bt0[kp][:, :, lo : lo + w]
            return btiles[ci2][:, kp, :, lo : lo + w]

        def chalves(W):
            out_, lo = [], 0
            while lo < W:
                out_.append((lo, min(512, W - lo)))
                lo += 512
            return out_

        # process full chunks in pairs: each lhsT load feeds 4 consecutive
        # matmuls, and the pair's 4 K=4 augment matmuls pack into ONE
        # concurrent 4-way row-group slot
        pairs = []
        ci = 0
        while ci < NCH:
            if (
                ci > 0  # chunk 0 alone: its DMA gates kernel start
                and ci + 1 < NCH
                and len(chalves(chunks[ci][1]))
                + len(chalves(chunks[ci + 1][1]))
                <= 4  # augments must fit the 4 PE row groups
            ):
                pairs.append((ci, ci + 1))
                ci += 2
            else:
                pairs.append((ci,))
                ci += 1

        for pair in pairs:
            for m in range(MT):
                pts = {}
                for ci2 in pair:
                    pts[ci2] = ppool.tile([P, 1024], f32, tag="pt", name=f"pt{ci2}_{m}")
                for kp in range(KTP):
                    for ci2 in pair:
                        c0, W = chunks[ci2]
                        for hlo, hw in chalves(W):
                            nc.tensor.matmul(
                                pts[ci2][:, hlo : hlo + hw],
                                lhsT=ftiles[kp][:, :, m * P : (m + 1) * P],
                                rhs=bslice(ci2, kp, hlo, hw),
                                start=(kp == 0),
                                stop=False,
                                perf_mode=DR,
                            )
                j = 0
                for ci2 in pair:
                    c0, W = chunks[ci2]
                    for hlo, hw in chalves(W):
                        nc.tensor.matmul(
                            pts[ci2][:, hlo : hlo + hw],
                            lhsT=augl_t[
                                32 * j : 32 * j + 2, m * P : (m + 1) * P
                            ],
                            rhs=augr_t[
                                32 * j : 32 * j + 2, c0 + hlo : c0 + hlo + hw
                            ],
                            start=False,
                            stop=True,
                            tile_position=(32 * j, 0),
                        )
                        j += 1
                for ci2 in pair:
                    c0, W = chunks[ci2]
                    u = upool.tile([P, 1024], f32, tag="u")
                    nc.scalar.copy(u[:, :W], pts[ci2][:, :W])
                    nc.vector.max(
                        cand_tiles[m][:, ci2 * 8 : ci2 * 8 + 8], u[:, :W]
                    )

        for m in range(MT):
            nc.sync.dma_start(out[m * P : (m + 1) * P, :], cand_tiles[m][:])

    nc.finalize()
    return nc


def _split_bf16(x):
    """hi/lo bf16 split of a float32 vector: hi + lo ~= x to ~2^-17."""
    import ml_dtypes

    bf = ml_dtypes.bfloat16
    hi = x.astype(bf)
    lo = (x - hi.astype(np.float32)).astype(bf)
    return hi, lo


def _host_prep(features, memory_bank):
    """Shard + lay out inputs for the 8 cores."""
    import ml_dtypes

    bf = ml_dtypes.bfloat16
    f8 = ml_dtypes.float8_e4m3
    B, D = features.shape
    N = memory_bank.shape[0]
    NSH = -(-N // NCORES)
    NPAD = max(NSH, 1024)
    if NPAD % 1024 and NPAD % 1024 < 8:
        NPAD = _ceil_to(NPAD, 1024)  # keep the ragged tail MAX8-legal (>=8)

    fT = np.ascontiguousarray(features.T).astype(f8)
    x_sq = np.einsum("bd,bd->b", features, features, dtype=np.float32)

    msq = np.einsum("nd,nd->n", memory_bank, memory_bank, dtype=np.float32)

    in_maps = []
    for i in range(NCORES):
        lo = i * NSH
        hi = min(lo + NSH, N)
        n_i = hi - lo
        if n_i == NPAD:
            bT = np.ascontiguousarray(memory_bank[lo:hi].T).astype(f8)
        else:
            bT = np.zeros((D, NPAD), f8)
            bT[:, :n_i] = memory_bank[lo:hi].T.astype(f8)
        mh, ml = _split_bf16(-0.5 * msq[lo:hi])
        augR = np.zeros((2, NPAD), bf)
        augR[0] = PAD_VAL
        augR[0, :n_i] = mh
        augR[1, :n_i] = ml
        in_maps.append({"f_t": fT, "b_t": bT, "aug_r": augR})
    return in_maps, NPAD, x_sq, msq


# test.py can flip these to get a profiled run
TRACE = False
LAST_RESULT = None
N_RECOMPUTED = 0


def _install_ntff_hook():
    """This container's `antenv` lacks `axon_hooks`; synthesize it so
    run_bass_kernel_spmd(trace=True) can profile via the axon .so."""
    import sys as _sys

    if "antenv.axon_hooks" in _sys.modules:
        return
    import contextlib, ctypes, types

    mod = types.ModuleType("antenv.axon_hooks")
    mod._hook = None
    mod.set_axon_ntff_profile_hook = lambda h: setattr(mod, "_hook", h)
    mod.get_axon_ntff_profile_hook = lambda: mod._hook

    so_path = "/opt/axon/libaxon_pjrt.so"
    try:
        lib = ctypes.CDLL(so_path)
        lib.axon_start_nrt_profile.argtypes = [
            ctypes.POINTER(ctypes.c_int64),
            ctypes.c_size_t,
        ]
        lib.axon_start_nrt_profile.restype = ctypes.c_int64
        lib.axon_stop_nrt_profile.argtypes = [ctypes.c_char_p]
        lib.axon_stop_nrt_profile.restype = ctypes.c_int64

        @contextlib.contextmanager
        def _hook(output_dir, device_ids):
            import jax

            jax.devices()
            if device_ids:
                ids = (ctypes.c_int64 * len(device_ids))(*device_ids)
                rc = lib.axon_start_nrt_profile(ids, len(device_ids))
            else:
                rc = lib.axon_start_nrt_profile(None, 0)
            if rc != 0:
                raise RuntimeError(f"axon_start_nrt_profile rc={rc}")
            try:
                yield
            finally:
                n = lib.axon_stop_nrt_profile(str(output_dir).encode())
                print(f"profile: {n} file(s) written to {output_dir}")

        mod._hook = _hook
    except (OSError, AttributeError):
        pass

    import antenv

    _sys.modules["antenv.axon_hooks"] = mod
    antenv.axon_hooks = mod


def _exact_row_scores(features, memory_bank, rows, kk):
    """Exact numpy top-k mean distance for a few suspect rows."""
    f = features[rows]  # [R, D]
    d2 = (
        np.einsum("rd,rd->r", f, f)[:, None]
        + np.einsum("nd,nd->n", memory_bank, memory_bank)[None, :]
        - 2.0 * (f @ memory_bank.T)
    )
    d2k = np.sort(d2, axis=1)[:, :kk]
    return np.sqrt(np.maximum(d2k, 0.0)).mean(axis=1)


def kernel(features, memory_bank, k):
    global LAST_RESULT, N_RECOMPUTED
    from concourse.bass_utils import run_bass_kernel_spmd

    features = np.asarray(features, dtype=np.float32)
    memory_bank = np.asarray(memory_bank, dtype=np.float32)
    B, D = features.shape
    N = memory_bank.shape[0]
    kk = min(int(k), N)
    if kk <= 0:
        # mean over an empty candidate set (matches jnp.mean of empty)
        return np.full(B, np.nan, np.float32)

    in_maps, NPAD, x_sq, msq = _host_prep(features, memory_bank)
    nc = _build(B, D, NPAD)

    if TRACE:
        _install_ntff_hook()
    res = run_bass_kernel_spmd(nc, in_maps, list(range(NCORES)), trace=TRACE)
    LAST_RESULT = res

    # gather per-(core, block) top-8 candidates; larger v = closer
    # (v = f.m - |m|^2/2, so d^2 = x_sq - 2 v)
    v = np.concatenate(
        [res.results[i]["cand"] for i in range(NCORES)], axis=1
    )  # [B, NCORES * 8 * nblocks]
    return _finalize(v, x_sq, features, memory_bank, kk)


def _finalize(v, x_sq, features, memory_bank, kk):
    """Reduce the per-(core, block) top-8 candidates to the final scores."""
    global N_RECOMPUTED
    kk_c = min(kk, v.shape[1])
    order = np.argsort(-v, axis=1)[:, :kk_c]  # observed top-k candidates
    vk = np.take_along_axis(v, order, axis=1)
    d = np.sqrt(np.maximum(x_sq[:, None] - 2.0 * vk, 0.0))
    scores = d.mean(axis=1).astype(np.float32)

    # A true top-k member can only be missing if >=8 elements of its
    # 1024-column block outrank it; then >=8 of the observed top-k come
    # from that block (index group of 8).  Recompute such rows exactly.
    N_RECOMPUTED = 0
    if kk >= 9:
        if kk > v.shape[1]:  # more than the candidate pool: all rows exact
            suspects = np.arange(v.shape[0])
        else:
            grp = np.sort(order // 8, axis=1)
            same8 = (grp[:, 7:] == grp[:, : grp.shape[1] - 7]).any(axis=1)
            suspects = np.nonzero(same8)[0]
        if suspects.size:
            N_RECOMPUTED = suspects.size
            scores[suspects] = _exact_row_scores(
                features, memory_bank, suspects, kk
            ).astype(np.float32)

    return scores


# revision 12
# speedup vs baseline: 1.2661x; 1.0141x over previous
"""KNN anomaly-score kernel for Trainium2 (8 NeuronCores, Bass/Tile).

Problem: features [B=1024, D=768], memory_bank [N=50000, D=768], k=9.
anomaly_score[b] = mean of the k smallest Euclidean distances from
features[b] to the memory bank rows.

Strategy (per the sharding hint): shard memory-bank rows across the 8
cores.  Each core computes its [B, N/8] block of v = f.m - |m|^2/2 on
the TensorEngine: the GEMM runs in fp8-e4m3 with the DoubleRow perf
mode (two K=128 subtiles reduced per instruction -> 2x bf16
throughput), while the bank-norm term is folded in exactly via a K=2
bf16 augmented matmul whose constants are split hi/lo across two bf16
rows (compensated summation), accumulated in fp32 PSUM.  The |f|^2 term
is a per-row constant - it cannot change the per-row selection - so the
host folds it in exactly afterwards (d^2 = x_sq - 2v).  The augment
matmuls of a chunk pair run concurrently in disjoint 32-row PE groups
(tile_position), so they cost ~one matmul slot; their lhs is an on-chip
memset of ones and their rhs replicas ride the idle GPSIMD DMA queue,
keeping the critical sync-queue DMAs (features + bank) uninterrupted.

Selection: for each 1024-column block the DVE MAX8 instruction extracts
the block's top-8 v values (one pass, no match_replace).  The device
returns all block candidates [B, 8*nblocks]; the host gathers the 8
cores' candidates and reduces to the global top-k.  A true top-k member
can be missing only if >=8 elements of its block rank above it, which
forces >=8 of the observed top-k to come from that single block - the
host detects exactly that condition and recomputes the affected rows
with numpy, so gross selection failures are corrected for any k.
"""

import functools
import sys

sys.path.insert(0, "/opt/trn_rl_repo")

import numpy as np

P = 128
NCORES = 8
PAD_VAL = -1.0e30  # v-value of padding columns (never selected)


def _ceil_to(x, m):
    return (x + m - 1) // m * m


@functools.lru_cache(maxsize=4)
def _build(B, D, NPAD):
    """Build (and finalize) the SPMD Bass module for one core's shard."""
    from contextlib import ExitStack

    import concourse.tile as tile
    from concourse import bacc, mybir

    f32 = mybir.dt.float32
    bf16 = mybir.dt.bfloat16
    fp8 = mybir.dt.float8e4
    DR = mybir.MatmulPerfMode.DoubleRow

    KT = D // P
    MT = B // P
    assert D % P == 0 and B % P == 0 and NPAD >= 1024
    assert KT % 2 == 0, "DoubleRow needs an even number of K tiles"
    KTP = KT // 2
    # process blocks of 1024 columns (one 2-bank PSUM tile), ragged tail
    chunks = []
    c0 = 0
    while c0 < NPAD:
        w = min(1024, NPAD - c0)
        rem = NPAD - c0 - w
        if 0 < rem < 8:
            w -= 8 - rem  # keep the next (last) chunk MAX8-legal (>=8)
        chunks.append((c0, w))
        c0 += w
    NCH = len(chunks)
    CW = 8 * NCH  # candidates per row per core

    nc = bacc.Bacc(
        "TRN2", target_bir_lowering=False, debug=False, num_devices=NCORES
    )

    f_t = nc.declare_dram_parameter("f_t", [D, B], fp8, isOutput=False)
    b_t = nc.declare_dram_parameter("b_t", [D, NPAD], fp8, isOutput=False)
    aug_r = nc.declare_dram_parameter("aug_r", [2, NPAD], bf16, isOutput=False)
    out = nc.declare_dram_parameter("cand", [B, CW], f32, isOutput=True)

    with tile.TileContext(nc) as tc, ExitStack() as ctx:
        cpool = ctx.enter_context(tc.tile_pool(name="const", bufs=1))
        bpool = ctx.enter_context(tc.tile_pool(name="bank", bufs=6))
        ppool = ctx.enter_context(tc.tile_pool(name="psum", bufs=4, space="PSUM"))
        upool = ctx.enter_context(tc.tile_pool(name="u", bufs=6))

        # group K tiles in pairs: DoubleRow consumes [K, 2, .] slices
        b_t_view = b_t.rearrange("(kp two p) n -> p kp two n", p=P, two=2)
        f_t_view = f_t.rearrange("(kp two p) b -> p kp two b", p=P, two=2)

        # PE warm-up during the initial DMA wait: garbage matmuls on a
        # zeroed tile get the HAM clock-gate to 2.4GHz before real work
        warm = cpool.tile([P, 512], bf16, tag="warm")
        nc.vector.memset(warm[:], 0.0)
        wpsum = ppool.tile([P, 1024], f32, tag="pt")  # borrow a pt slot
        for _ in range(5):
            nc.tensor.matmul(
                wpsum[:, :512], lhsT=warm[:, :P], rhs=warm[:], start=True, stop=True
            )

        # per-ktp tiles + interleaved DMAs so the first matmuls can start
        # as soon as the ktp=0 slices land (instead of after one huge DMA)
        ftiles = [
            cpool.tile([P, 2, B], fp8, tag=f"ft{kp}", name=f"ft{kp}")
            for kp in range(KTP)
        ]
        bt0 = [
            bpool.tile([P, 2, 1024], fp8, tag=f"bt0_{kp}", name=f"bt0_{kp}")
            for kp in range(KTP)
        ]
        augl_t = cpool.tile([P, B], bf16, tag="augl")
        augr_t = cpool.tile([P, NPAD], bf16, tag="augr")
        W0 = chunks[0][1]
        # first-matmul gate: split bt0[0] so the first half-chunk's 384KB
        # lands (and the PE starts) ~1.5us sooner
        h0 = min(512, W0)
        nc.sync.dma_start(bt0[0][:, :, :h0], b_t_view[:, 0, :, :h0])
        nc.sync.dma_start(ftiles[0][:], f_t_view[:, 0, :, :])
        if W0 > h0:
            nc.sync.dma_start(bt0[0][:, :, h0:W0], b_t_view[:, 0, :, h0:W0])
        for kp in range(1, KTP):
            nc.sync.dma_start(bt0[kp][:, :, :W0], b_t_view[:, kp, :, :W0])
            nc.sync.dma_start(ftiles[kp][:], f_t_view[:, kp, :, :])
        # augment rows replicated at partition bases {0,32,64,96} so four
        # K=2 augment matmuls can run concurrently in disjoint PE row groups.
        # The ones-vector lhs is memset on-chip (no DMA).  The rhs replica
        # transfers are latency-bound (~2us each, serializing to ~19us on a
        # single queue), so spread them across the three idle engine queues
        # and keep the critical sync-queue stream (features + bank chunks)
        # uninterrupted.
        for j in range(4):
            nc.gpsimd.memset(augl_t[32 * j : 32 * j + 2, :], 1.0)
        aug_qs = [nc.gpsimd, nc.scalar, nc.gpsimd, nc.scalar]
        for j in range(4):
            aug_qs[j].dma_start(augr_t[32 * j : 32 * j + 2, :], aug_r[:])

        # enqueue every later bank chunk's load up front; the pool's slots
        # gate the actual transfers
        btiles = {}
        for ci, (c0, W) in enumerate(chunks):
            if ci == 0:
                continue
            btile = bpool.tile([P, KTP, 2, 1024], fp8, tag="bt", name=f"bt{ci}")
            nc.sync.dma_start(btile[:, :, :, :W], b_t_view[:, :, :, c0 : c0 + W])
            btiles[ci] = btile

        cand_tiles = [
            cpool.tile([P, CW], f32, tag=f"cand{m}", name=f"cand{m}")
            for m in range(MT)
        ]

        def bslice(ci2, kp, lo, w):
            if ci2 == 0:
                return bt0[kp][:, :, lo : lo + w]
            return btiles[ci2][:, kp, :, lo : lo + w]

        def chalves(W):
            out_, lo = [], 0
            while lo < W:
                out_.append((lo, min(512, W - lo)))
                lo += 512
            return out_

        # process full chunks in pairs: each lhsT load feeds 4 consecutive
        # matmuls, and the pair's 4 K=4 augment matmuls pack into ONE
        # concurrent 4-way row-group slot
        pairs = []
        ci = 0
        while ci < NCH:
            if (
                ci > 0  # chunk 0 alone: its DMA gates kernel start
                and ci + 1 < NCH
                and len(chalves(chunks[ci][1]))
                + len(chalves(chunks[ci + 1][1]))
                <= 4  # augments must fit the 4 PE row groups
            ):
                pairs.append((ci, ci + 1))
                ci += 2
            else:
                pairs.append((ci,))
                ci += 1

        for pair in pairs:
            for m in range(MT):
                pts = {}
                for ci2 in pair:
                    pts[ci2] = ppool.tile([P, 1024], f32, tag="pt", name=f"pt{ci2}_{m}")
                for kp in range(KTP):
                    for ci2 in pair:
                        c0, W = chunks[ci2]
                        for hlo, hw in chalves(W):
                            nc.tensor.matmul(
                                pts[ci2][:, hlo : hlo + hw],
                                lhsT=ftiles[kp][:, :, m * P : (m + 1) * P],
                                rhs=bslice(ci2, kp, hlo, hw),
                                start=(kp == 0),
                                stop=False,
                                perf_mode=DR,
                            )
                j = 0
                for ci2 in pair:
                    c0, W = chunks[ci2]
                    for hlo, hw in chalves(W):
                        nc.tensor.matmul(
                            pts[ci2][:, hlo : hlo + hw],
                            lhsT=augl_t[
                                32 * j : 32 * j + 2, m * P : (m + 1) * P
                            ],
                            rhs=augr_t[
                                32 * j : 32 * j + 2, c0 + hlo : c0 + hlo + hw
                            ],
                            start=False,
                            stop=True,
                            tile_position=(32 * j, 0),
                        )
                        j += 1
                for ci2 in pair:
                    c0, W = chunks[ci2]
                    u = upool.tile([P, 1024], f32, tag="u")
                    nc.scalar.copy(u[:, :W], pts[ci2][:, :W])
                    nc.vector.max(
                        cand_tiles[m][:, ci2 * 8 : ci2 * 8 + 8], u[:, :W]
                    )

        for m in range(MT):
            nc.sync.dma_start(out[m * P : (m + 1) * P, :], cand_tiles[m][:])

    nc.finalize()
    return nc


def _split_bf16(x):
    """hi/lo bf16 split of a float32 vector: hi + lo ~= x to ~2^-17."""
    import ml_dtypes

    bf = ml_dtypes.bfloat16
    hi = x.astype(bf)
    lo = (x - hi.astype(np.float32)).astype(bf)
    return hi, lo


def _host_prep(features, memory_bank):
    """Shard + lay out inputs for the 8 cores."""
    import ml_dtypes

    bf = ml_dtypes.bfloat16
    f8 = ml_dtypes.float8_e4m3
    B, D = features.shape
    N = memory_bank.shape[0]
    NSH = -(-N // NCORES)
    NPAD = max(NSH, 1024)
    if NPAD % 1024 and NPAD % 1024 < 8:
        NPAD = _ceil_to(NPAD, 1024)  # keep the ragged tail MAX8-legal (>=8)

    fT = np.ascontiguousarray(features.T).astype(f8)
    x_sq = np.einsum("bd,bd->b", features, features, dtype=np.float32)

    msq = np.einsum("nd,nd->n", memory_bank, memory_bank, dtype=np.float32)

    in_maps = []
    for i in range(NCORES):
        lo = i * NSH
        hi = min(lo + NSH, N)
        n_i = hi - lo
        if n_i == NPAD:
            bT = np.ascontiguousarray(memory_bank[lo:hi].T).astype(f8)
        else:
            bT = np.zeros((D, NPAD), f8)
            bT[:, :n_i] = memory_bank[lo:hi].T.astype(f8)
        mh, ml = _split_bf16(-0.5 * msq[lo:hi])
        augR = np.zeros((2, NPAD), bf)
        augR[0] = PAD_VAL
        augR[0, :n_i] = mh
        augR[1, :n_i] = ml
        in_maps.append({"f_t": fT, "b_t": bT, "aug_r": augR})
    return in_maps, NPAD, x_sq, msq


# test.py can flip these to get a profiled run
TRACE = False
LAST_RESULT = None
N_RECOMPUTED = 0


def _install_ntff_hook():
    """This container's `antenv` lacks `axon_hooks`; synthesize it so
    run_bass_kernel_spmd(trace=True) can profile via the axon .so."""
    import sys as _sys

    if "antenv.axon_hooks" in _sys.modules:
        return
    import contextlib, ctypes, types

    mod = types.ModuleType("antenv.axon_hooks")
    mod._hook = None
    mod.set_axon_ntff_profile_hook = lambda h: setattr(mod, "_hook", h)
    mod.get_axon_ntff_profile_hook = lambda: mod._hook

    so_path = "/opt/axon/libaxon_pjrt.so"
    try:
        lib = ctypes.CDLL(so_path)
        lib.axon_start_nrt_profile.argtypes = [
            ctypes.POINTER(ctypes.c_int64),
            ctypes.c_size_t,
        ]
        lib.axon_start_nrt_profile.restype = ctypes.c_int64
        lib.axon_stop_nrt_profile.argtypes = [ctypes.c_char_p]
        lib.axon_stop_nrt_profile.restype = ctypes.c_int64

        @contextlib.contextmanager
        def _hook(output_dir, device_ids):
            import jax

            jax.devices()
            if device_ids:
                ids = (ctypes.c_int64 * len(device_ids))(*device_ids)
                rc = lib.axon_start_nrt_profile(ids, len(device_ids))
            else:
                rc = lib.axon_start_nrt_profile(None, 0)
            if rc != 0:
                raise RuntimeError(f"axon_start_nrt_profile rc={rc}")
            try:
                yield
            finally:
                n = lib.axon_stop_nrt_profile(str(output_dir).encode())
                print(f"profile: {n} file(s) written to {output_dir}")

        mod._hook = _hook
    except (OSError, AttributeError):
        pass

    import antenv

    _sys.modules["antenv.axon_hooks"] = mod
    antenv.axon_hooks = mod


def _exact_row_scores(features, memory_bank, rows, kk):
    """Exact numpy top-k mean distance for a few suspect rows."""
    f = features[rows]  # [R, D]
    d2 = (
        np.einsum("rd,rd->r", f, f)[:, None]
        + np.einsum("nd,nd->n", memory_bank, memory_bank)[None, :]
        - 2.0 * (f @ memory_bank.T)
    )
    d2k = np.sort(d2, axis=1)[:, :kk]
    return np.sqrt(np.maximum(d2k, 0.0)).mean(axis=1)


def kernel(features, memory_bank, k):
    global LAST_RESULT, N_RECOMPUTED
    from concourse.bass_utils import run_bass_kernel_spmd

    features = np.asarray(features, dtype=np.float32)
    memory_bank = np.asarray(memory_bank, dtype=np.float32)
    B, D = features.shape
    N = memory_bank.shape[0]
    kk = min(int(k), N)
    if kk <= 0:
        # mean over an empty candidate set (matches jnp.mean of empty)
        return np.full(B, np.nan, np.float32)

    in_maps, NPAD, x_sq, msq = _host_prep(features, memory_bank)
    nc = _build(B, D, NPAD)

    if TRACE:
        _install_ntff_hook()
    res = run_bass_kernel_spmd(nc, in_maps, list(range(NCORES)), trace=TRACE)
    LAST_RESULT = res

    # gather per-(core, block) top-8 candidates; larger v = closer
    # (v = f.m - |m|^2/2, so d^2 = x_sq - 2 v)
    v = np.concatenate(
        [res.results[i]["cand"] for i in range(NCORES)], axis=1
    )  # [B, NCORES * 8 * nblocks]
    return _finalize(v, x_sq, features, memory_bank, kk)


def _finalize(v, x_sq, features, memory_bank, kk):
    """Reduce the per-(core, block) top-8 candidates to the final scores."""
    global N_RECOMPUTED
    kk_c = min(kk, v.shape[1])
    order = np.argsort(-v, axis=1)[:, :kk_c]  # observed top-k candidates
    vk = np.take_along_axis(v, order, axis=1)
    d = np.sqrt(np.maximum(x_sq[:, None] - 2.0 * vk, 0.0))
    scores = d.mean(axis=1).astype(np.float32)

    # A true top-k member can only be missing if >=8 elements of its
    # 1024-column block outrank it; then >=8 of the observed top-k come
    # from that block (index group of 8).  Recompute such rows exactly.
    N_RECOMPUTED = 0
    if kk >= 9:
        if kk > v.shape[1]:  # more than the candidate pool: all rows exact
            suspects = np.arange(v.shape[0])
        else:
            grp = np.sort(order // 8, axis=1)
            same8 = (grp[:, 7:] == grp[:, : grp.shape[1] - 7]).any(axis=1)
            suspects = np.nonzero(same8)[0]
        if suspects.size:
            N_RECOMPUTED = suspects.size
            scores[suspects] = _exact_row_scores(
                features, memory_bank, suspects, kk
            ).astype(np.float32)

    return scores
